# revision 49
# baseline (speedup 1.0000x reference)
"""CrossAttentionFusion Trainium2 kernel — fp8 DoubleRow edition.

Full inputs -> shard (batch x query-half) over 8 NeuronCores -> full output.

Per core (batch b = core//2, query half = core%2): NH=2048 queries n,
N=4096 keys m, C=256 channels.

Host precompute (exact f32, then fp8e4 quantization):
  Q'[c,n] = (q_w^T k_w)^T x1 + k_w^T q_b        (logits rhs)
  Y'[o,m] = G[o] * (p_w v_w x2)[o,m] + Bc[o]    (fusion rhs; G = BN scale,
            Bc folded in: P/S + Bc == (P + Bc*S)/S and the ones column
            provides S, so the BN bias rides the matmul for free)
  gate[n] = sigmoid(gate_w [x1;x2] + gate_b)    (per-query scalar)
Device per 512-query block j:
  L[m,n]  = x2^T Q'      fp8 DoubleRow matmuls (contraction c=256/instr)
  E       = exp(L/16 - 2.5), fp8e4 out (offset cancels in P'/S); ~10
            slots/block on ACT (exp table), ~6 on DVE via Schraudolph in
            the fp8 bit domain (one saturating tensor_scalar to uint8)
  P'[n,o] = sum_m E[m,n] Y'[o,m]  fp8 DR, [query, channel] layout;
            ones column -> P'[:,256] = S
  out^T   = x1^T + relu(P'[:, :256] * (gate/S))   (g>0 lets the gate fold
            inside the relu; per-partition scalars only: recip, 2 muls)
Scheduling: fusion runs in two nt-pair passes (A = nt 0,1 in-block;
B = nt 2,3 drains early next block off the still-live e8 tiles) so psF
fits in 2 PSUM banks and psL gets a 3-deep ring — the exp(k) ->
logits(k+2) -> exp(k+2) PSUM-WAR cycle then never paces the PE, which
runs at ~95% MM occupancy.  Emission is strictly pass-ordered (the psF
bank chain would deadlock the in-order PE queue otherwise).  Posts fire
per-nt straight off PSUM; output rows collect per block and leave as one
DMA.  Startup DMAs spread over the sync/gpsimd/scalar SWDGE rings (~2us
cold-start each) ordered by first use; 12 warmup matmuls hold the PE
p-state until real work lands.
"""
from contextlib import ExitStack

import numpy as np
import ml_dtypes

import concourse.bass as bass
import concourse.mybir as mybir
import concourse.tile as tile
from concourse import bacc
from concourse.bass_utils import run_bass_kernel_spmd

F32 = mybir.dt.float32
BF16 = mybir.dt.bfloat16
FP8 = mybir.dt.float8e4
AF = mybir.ActivationFunctionType
OP = mybir.AluOpType
DR = mybir.MatmulPerfMode.DoubleRow
NP8 = ml_dtypes.float8_e4m3

B, C, H, W = 4, 256, 64, 64
N = H * W            # 4096 keys per batch
NCORES = 8
NH = N // 2          # 2048 queries per core
NBLK = 512           # query block
NBLOCKS = NH // NBLK
MT = N // 128        # 32 key tiles
MT2 = MT // 2        # 16 DoubleRow key-pair steps
YW = 272             # yt row: 256 channels + ones col + pad to %16
NT = NBLK // 128     # 4 query tiles per block
EPS = 1e-5
SCALE = float(C) ** -0.5
EOFF = 2.5           # exp offset; cancels in P'/S


def build():
    nc = bacc.Bacc("TRN2", target_bir_lowering=False, debug=False,
                   num_devices=NCORES)
    q8_d = nc.dram_tensor("q8", [128, 2 * NH], FP8, kind="ExternalInput")
    x2_d = nc.dram_tensor("x2dr", [128, 2 * N], FP8, kind="ExternalInput")
    yt_d = nc.dram_tensor("yt", [128, MT * YW], FP8, kind="ExternalInput")
    x1_d = nc.dram_tensor("x1t", [NH, C], BF16, kind="ExternalInput")
    gc_d = nc.dram_tensor("gatec", [128, NBLOCKS * NT], F32,
                          kind="ExternalInput")
    out_d = nc.dram_tensor("out", [NH, C], BF16, kind="ExternalOutput")

    with tile.TileContext(nc, pool_alloc_mode="queue") as tc, ExitStack() as ctx:
        pers = ctx.enter_context(tc.tile_pool(name="pers", bufs=1))
        epool = ctx.enter_context(tc.tile_pool(name="epool", bufs=2))
        work = ctx.enter_context(tc.tile_pool(name="work", bufs=2))
        outs = ctx.enter_context(tc.tile_pool(name="outs", bufs=2))
        psL = ctx.enter_context(tc.tile_pool(name="psL", bufs=3, space="PSUM"))
        psF = ctx.enter_context(tc.tile_pool(name="psF", bufs=2, space="PSUM"))

        q8b = [pers.tile([128, 2, NBLK], FP8, tag=f"q8b{j}", name=f"q8b{j}")
               for j in range(NBLOCKS)]
        # x2 chunks: small leading chunks (block-0 compute starts as soon as
        # its chunk lands — dep tracking is whole-tile) then two big ones
        # (fewer DMA triggers => fewer semaphores => shorter epilogue)
        x2s = [pers.tile([128, 2, 512], FP8, tag=f"x2s{c}", name=f"x2s{c}")
               for c in range(2)]
        x2b = [pers.tile([128, 2, 1536], FP8, tag=f"x2b{c}", name=f"x2b{c}")
               for c in range(2)]
        yts = [pers.tile([128, 4, YW], FP8, tag=f"yts{i}", name=f"yts{i}")
               for i in range(2)]
        ytb = [pers.tile([128, 6, YW], FP8, tag=f"ytb{i}", name=f"ytb{i}")
               for i in range(4)]
        x1ps = pers.tile([128, 4, C], BF16, tag="x1ps", name="x1ps")
        x1pb = pers.tile([128, NBLOCKS * NT - 4, C], BF16, tag="x1pb",
                         name="x1pb")
        gc = pers.tile([128, NBLOCKS * NT], F32, tag="gc", name="gc")
        nbias = pers.tile([128, 1], F32, tag="nbias", name="nbias")

        def _x1p(t):
            return x1ps[:, t, :] if t < 4 else x1pb[:, t - 4, :]

        def _x2ap(mt):
            # [128, 2, 128] lhsT slice for key tile mt
            if mt < 8:
                return x2s[mt // 4][:, :, (mt % 4) * 128:(mt % 4 + 1) * 128]
            c, o = (0, mt - 8) if mt < 20 else (1, mt - 20)
            return x2b[c][:, :, o * 128:(o + 1) * 128]

        def _ytap(k):
            # [128, 2, 257] rhs slice for fusion step k (key pair 2k,2k+1)
            t = 2 * k
            if t < 8:
                tile, o = yts[t // 4], t % 4
            else:
                tile, o = ytb[(t - 8) // 6], (t - 8) % 6
            return tile[:, o:o + 2, 0:257]

        def _x2dma(tile, c0, w):
            nc.sync.dma_start(tile[:, 0, :], x2_d[:, c0:c0 + w])
            nc.gpsimd.dma_start(tile[:, 1, :], x2_d[:, N + c0:N + c0 + w])

        def _ytdma(tile, t0, tn, eng):
            eng.dma_start(
                tile[:],
                yt_d[:, t0 * YW:(t0 + tn) * YW].rearrange(
                    "p (t y) -> p t y", y=YW))

        def _q8(j):
            js = slice(j * NBLK, (j + 1) * NBLK)
            nc.sync.dma_start(q8b[j][:, 0, :], q8_d[:, js])
            nc.gpsimd.dma_start(q8b[j][:, 1, :],
                                q8_d[:, NH + j * NBLK:NH + (j + 1) * NBLK])

        with nc.named_scope("pre"):
            # critical-path DMAs first, spread over three engine rings
            # (only sync/gpsimd/scalar can initiate DMAs): each SWDGE ring
            # has ~2us cold-start, and per-ring FIFO order matches first
            # use (x2s0+q8b0 gate the first logits, yts0 gates fusion k=0)
            nc.sync.dma_start(x2s[0][:, 0, :], x2_d[:, 0:512])
            nc.gpsimd.dma_start(x2s[0][:, 1, :], x2_d[:, N:N + 512])
            nc.scalar.dma_start(q8b[0][:, 0, :], q8_d[:, 0:NBLK])
            nc.sync.dma_start(q8b[0][:, 1, :], q8_d[:, NH:NH + NBLK])
            nc.gpsimd.dma_start(x2s[1][:, 0, :], x2_d[:, 512:1024])
            nc.scalar.dma_start(x2s[1][:, 1, :], x2_d[:, N + 512:N + 1024])
            _ytdma(yts[0], 0, 4, nc.scalar)
            # PE warmup: ramp the clock out of the low p-state on scratch
            # data while the first DMAs land; memsets on the (idle) DVE
            # queue so they don't delay the DMA triggers.
            wl = pers.tile([128, 2, 128], FP8, tag="wl", name="wl")
            wr = pers.tile([128, 2, 257], FP8, tag="wr", name="wr")
            nc.vector.memset(wl[:], 0.0)
            nc.vector.memset(wr[:], 0.0)
            nc.vector.memset(nbias[:], -EOFF)
            for w in range(12):
                wp = psF.tile([128, 257], F32, tag="fuse", name="fuse")
                nc.tensor.matmul(wp[:], wl[:], wr[:], start=True, stop=True,
                                 perf_mode=DR)

        def dma_feed():
            # Deferred DMA issues, a step per slot, just ahead of need;
            # per-ring FIFO order tracks each tile's first-use slot.
            nc.sync.dma_start(x2b[0][:, 0, :], x2_d[:, 1024:2560])
            nc.gpsimd.dma_start(x2b[0][:, 1, :], x2_d[:, N + 1024:N + 2560])
            yield
            _ytdma(ytb[0], 8, 6, nc.sync)
            _ytdma(yts[1], 4, 4, nc.gpsimd)
            yield
            nc.sync.dma_start(x2b[1][:, 0, :], x2_d[:, 2560:4096])
            _ytdma(ytb[1], 14, 6, nc.gpsimd)
            yield
            _ytdma(ytb[2], 20, 6, nc.sync)
            nc.gpsimd.dma_start(x2b[1][:, 1, :], x2_d[:, N + 2560:N + 4096])
            yield
            nc.sync.dma_start(q8b[1][:, 0, :], q8_d[:, NBLK:2 * NBLK])
            _ytdma(ytb[3], 26, 6, nc.gpsimd)
            yield
            nc.gpsimd.dma_start(q8b[1][:, 1, :],
                                q8_d[:, NH + NBLK:NH + 2 * NBLK])
            nc.sync.dma_start(
                x1ps[:], x1_d[0:512, :].rearrange("(t p) c -> p t c", p=128))
            yield
            nc.gpsimd.dma_start(gc[:], gc_d[:])
            yield
            nc.sync.dma_start(
                x1pb[:], x1_d[512:NH, :].rearrange("(t p) c -> p t c", p=128))
            _q8(2)
            yield
            _q8(3)
            yield

        feed = dma_feed()

        obufs = {}
        pcnt = {}

        def emit_post_nt(j, nt, fuse_nt):
            # per-nt postlude, straight off PSUM: ginv=gate/S,
            # rg=relu(P'*ginv), out=rg+x1.  (BN bias already in P' via Y'.)
            # Output rows collect in a per-block buffer; one DMA per block
            # (fewer triggers and live DMA semaphores).
            t_idx = j * NT + nt
            with nc.named_scope(f"post{j}_{nt}"):
                if j not in obufs:
                    obufs[j] = outs.tile([128, NT, C], BF16, tag="ob",
                                         name="ob")
                    pcnt[j] = 0
                ginv = work.tile([128, 1], F32, tag="ginv", name="ginv")
                nc.vector.reciprocal_approx_fast(ginv[:],
                                                 fuse_nt[:, 256:257])
                nc.vector.tensor_scalar(ginv[:], ginv[:],
                                        gc[:, t_idx:t_idx + 1], None,
                                        op0=OP.mult)
                rg = work.tile([128, C], F32, tag="rg", name="rg")
                nc.vector.tensor_scalar(rg[:], fuse_nt[:, 0:256], ginv[:],
                                        0.0, op0=OP.mult, op1=OP.max)
                nc.vector.scalar_tensor_tensor(
                    obufs[j][:, nt, :], rg[:], 0.0, _x1p(t_idx),
                    op0=OP.bypass, op1=OP.add)
                pcnt[j] += 1
                if pcnt[j] == NT:
                    eng = nc.sync if j % 2 == 0 else nc.gpsimd
                    eng.dma_start(
                        out_d[j * NBLK:(j + 1) * NBLK, :].rearrange(
                            "(t p) c -> p t c", p=128),
                        obufs[j][:])

        def emit_fusion(e8, nt, k, fuse, fcnt, j):
            if fcnt[nt] == 0:
                fuse[nt] = psF.tile([128, 257], F32, tag="fuse", name="fuse")
            ex = e8[k % 2]
            kk = (k // 2) * 2
            nc.tensor.matmul(
                fuse[nt][:, 0:257],
                ex[:, kk:kk + 2, nt * 128:(nt + 1) * 128],
                _ytap(k),
                start=(fcnt[nt] == 0), stop=(fcnt[nt] == MT2 - 1),
                perf_mode=DR)
            fcnt[nt] += 1
            if fcnt[nt] == MT2:
                emit_post_nt(j, nt, fuse[nt])

        # Hybrid exp, one instruction per slot.  ACT slots use the
        # activation table; DVE slots run Schraudolph directly in the
        # fp8e4m3 bit domain — one saturating tensor_scalar to uint8:
        # i8 = z*8/ln2 + (56 - c - EOFF*8/ln2).  Underflow clamps to 0
        # (= exp underflow, correct); overflow can't reach 255 for our
        # logit range.
        A_S = (8.0 / float(np.log(2.0))) * SCALE
        B_S = 56.0 - 0.30 - EOFF * (8.0 / float(np.log(2.0)))
        U8 = mybir.dt.uint8

        def emit_exp(e8, mt2, lp, on_act):
            ex = e8[mt2 % 2]
            kk = (mt2 // 2) * 2
            dst = ex[:, kk:kk + 2, :]
            if on_act:
                nc.scalar.activation(dst, lp[:], AF.Exp, scale=SCALE,
                                     bias=nbias[:])
            else:
                nc.vector.tensor_scalar(dst.bitcast(U8), lp[:], A_S, B_S,
                                        op0=OP.mult, op1=OP.add)

        # Two-pass fusion: with only 2 psF banks (pass A = nt 0,1 during
        # the block; pass B = nt 2,3 draining early next block off the
        # still-live e8 tiles), psL gets 3 buffers and the
        # exp(k) -> logits(k+2) -> exp(k+2) PSUM-WAR ring stops pacing the
        # kernel.  Emission is strictly pass-ordered (A(j) after B(j-1))
        # — the psF bank chain would deadlock the in-order PE queue
        # otherwise.
        CAP = 8
        LAG = 3
        fq = []
        avail_at = {}

        def pump(budget):
            while fq and budget > 0:
                e8x, nt, k, fusex, fcntx, jx = fq.pop(0)
                emit_fusion(e8x, nt, k, fusex, fcntx, jx)
                budget -= 1

        # DVE whole-slot exp schedule (rest on ACT).  Middle blocks keep
        # DVE free for slots 0-3 so the previous block's A-pass posts run
        # immediately and release the psF banks before pass B pops.
        DVE_KS = {0: (1, 3, 5, 7, 9, 11, 13),
                  NBLOCKS - 1: (4, 6, 8, 10, 12, 14)}
        DVE_MID = (4, 6, 8, 10, 12, 14)

        S = 0
        prevB = []
        for j in range(NBLOCKS):
            with nc.named_scope(f"blk{j}"):
                e8 = [epool.tile([128, MT2, NBLK], FP8, tag=f"E8{p}",
                                 name=f"E8{p}") for p in range(2)]
                fuse = {}
                fcnt = [0] * NT
                myB = []
                dve_ks = DVE_KS.get(j, DVE_MID)
                for mt2 in range(MT2):
                    # fusion first: logits stall on the psL ring (WAR vs
                    # exp(k-3)) and the in-order PE queue would
                    # head-of-line-block queued fusion work behind them
                    if mt2 == 2:
                        # all A(j-1) items are enqueued by slot 1 (the
                        # last two use lag 2); B(j-1) goes next (before
                        # this block's own A items), nt-major so its
                        # banks free early
                        prevB.sort(key=lambda it: (it[1], it[2]))
                        fq.extend(prevB)
                        prevB = []
                    for item in avail_at.pop(S, ()):
                        fq.append(item)
                    pump(CAP)
                    lp = psL.tile([128, 2, NBLK], F32, tag="L", name="L")
                    for sub in range(2):
                        nc.tensor.matmul(
                            lp[:, sub, :], _x2ap(2 * mt2 + sub), q8b[j][:],
                            start=True, stop=True, perf_mode=DR)
                    emit_exp(e8, mt2, lp, mt2 not in dve_ks)
                    lag = LAG if mt2 < 14 else 2
                    avail_at.setdefault(S + lag, []).extend(
                        (e8, nt, mt2, fuse, fcnt, j) for nt in (0, 1))
                    myB.extend((e8, nt, mt2, fuse, fcnt, j) for nt in (2, 3))
                    next(feed, None)
                    S += 1
                prevB = myB
        with nc.named_scope("tail"):
            # all exps are emitted; drain remaining A items then the last
            # B pass, nt-major so each nt's post (and its out-DMA) fires
            # as early as possible
            for items in avail_at.values():
                fq.extend(items)
            avail_at.clear()
            fq.sort(key=lambda it: (it[5], it[1], it[2]))
            prevB.sort(key=lambda it: (it[1], it[2]))
            fq.extend(prevB)
            pump(len(fq))
    nc.compile()
    return nc


_NC = None


def _get_nc():
    global _NC
    if _NC is None:
        _NC = build()
    return _NC


def kernel(**inputs):
    x1 = np.asarray(inputs["x1"], np.float32).reshape(B, C, N)
    x2 = np.asarray(inputs["x2"], np.float32).reshape(B, C, N)
    q_w = np.asarray(inputs["q_w"], np.float32)
    k_w = np.asarray(inputs["k_w"], np.float32)
    v_w = np.asarray(inputs["v_w"], np.float32)
    p_w = np.asarray(inputs["proj_w"], np.float32)
    q_b = np.asarray(inputs["q_b"], np.float32)
    v_b = np.asarray(inputs["v_b"], np.float32)
    p_b = np.asarray(inputs["proj_b"], np.float32)
    gamma = np.asarray(inputs["bn_gamma"], np.float32)
    beta = np.asarray(inputs["bn_beta"], np.float32)
    mean = np.asarray(inputs["bn_mean"], np.float32)
    var = np.asarray(inputs["bn_var"], np.float32)
    gate_w = np.asarray(inputs["gate_w"], np.float32)
    gate_b = np.asarray(inputs["gate_b"], np.float32)

    wqk = q_w.T @ k_w                      # [C,C]
    A = gamma[:, None] / np.sqrt(var + EPS)[:, None] * (p_w @ v_w)  # G*(pw vw)
    G = gamma / np.sqrt(var + EPS)
    Bc = (beta + (p_b + p_w @ v_b - mean) * G).astype(np.float32)
    qpb = (k_w.T @ q_b).astype(np.float32)

    in_maps = []
    for b in range(B):
        Qp = (wqk.T @ x1[b] + qpb[:, None]).astype(NP8)      # [C, N]
        Y8 = (A @ x2[b] + Bc[:, None]).astype(NP8)            # [C, N] + bias
        glog = gate_w[0, :C] @ x1[b] + gate_w[0, C:] @ x2[b] + gate_b[0]
        gate = (1.0 / (1.0 + np.exp(-glog))).astype(np.float32)  # [N]
        x28 = x2[b].astype(NP8)
        # x2dr/q8 layouts: [p, h, m] = arr[h*128+p, m]
        x2dr = np.ascontiguousarray(
            x28.reshape(2, 128, N).transpose(1, 0, 2).reshape(128, 2 * N))
        yt = np.zeros((128, MT, YW), NP8)
        yt[:, :, :C] = np.ascontiguousarray(
            Y8.reshape(C, MT, 128).transpose(2, 1, 0))
        yt[:, :, C] = np.float32(1.0)
        yt = np.ascontiguousarray(yt.reshape(128, MT * YW))
        for half in range(2):
            hq = slice(half * NH, (half + 1) * NH)
            q8 = np.ascontiguousarray(
                Qp[:, hq].reshape(2, 128, NH).transpose(1, 0, 2)
                .reshape(128, 2 * NH))
            x1t = np.ascontiguousarray(
                x1[b][:, hq].T.astype(ml_dtypes.bfloat16))    # [NH, C]
            gc = np.ascontiguousarray(
                gate[hq].reshape(NBLOCKS * NT, 128).T.astype(np.float32))
            in_maps.append({
                "q8": q8, "x2dr": x2dr, "yt": yt, "x1t": x1t,
                "gatec": gc,
            })

    nc = _get_nc()
    res = run_bass_kernel_spmd(nc, in_maps, core_ids=list(range(NCORES)))
    out = np.empty((B, C, N), np.float32)
    for core in range(NCORES):
        b, half = divmod(core, 2)
        out[b, :, half * NH:(half + 1) * NH] = \
            res.results[core]["out"].astype(np.float32).T
    return out.reshape(B, C, H, W)


# revision 52
# speedup vs baseline: 1.1918x; 1.1918x over previous
"""CrossAttentionFusion Trainium2 kernel — fp8 DoubleRow edition.

Full inputs -> shard (batch x query-half) over 8 NeuronCores -> full output.

Per core (batch b = core//2, query half = core%2): NH=2048 queries n,
N=4096 keys m, C=256 channels.

Host precompute (exact f32, then fp8e4 quantization):
  Q'[c,n] = (q_w^T k_w)^T x1 + k_w^T q_b        (logits rhs)
  Y'[o,m] = G[o] * (p_w v_w x2)[o,m] + Bc[o]    (fusion rhs; G = BN scale,
            Bc folded in: P/S + Bc == (P + Bc*S)/S and the ones column
            provides S, so the BN bias rides the matmul for free)
  gate[n] = sigmoid(gate_w [x1;x2] + gate_b)    (per-query scalar)
Device per 512-query block j:
  L[m,n]  = x2^T Q'      fp8 DoubleRow matmuls (contraction c=256/instr)
  E       = exp(L/16 - 2.5), fp8e4 out (offset cancels in P'/S); ~10
            slots/block on ACT (exp table), ~6 on DVE via Schraudolph in
            the fp8 bit domain (one saturating tensor_scalar to uint8)
  P'[n,o] = sum_m E[m,n] Y'[o,m]  fp8 DR, [query, channel] layout;
            ones column -> P'[:,256] = S
  out^T   = x1^T + relu(P'[:, :256] * (gate/S))   (g>0 lets the gate fold
            inside the relu; per-partition scalars only: recip, 2 muls)
Scheduling: fusion runs in two nt-pair passes (A = nt 0,1 in-block;
B = nt 2,3 drains early next block off the still-live e8 tiles) so psF
fits in 2 PSUM banks and psL gets a 3-deep ring — the exp(k) ->
logits(k+2) -> exp(k+2) PSUM-WAR cycle then never paces the PE, which
runs at ~95% MM occupancy.  Emission is strictly pass-ordered (the psF
bank chain would deadlock the in-order PE queue otherwise).  Posts fire
per-nt straight off PSUM; output rows collect per block and leave as one
DMA.  Startup DMAs spread over the sync/gpsimd/scalar SWDGE rings (~2us
cold-start each) ordered by first use; 12 warmup matmuls hold the PE
p-state until real work lands.
"""
from contextlib import ExitStack

import numpy as np
import ml_dtypes

import concourse.bass as bass
import concourse.mybir as mybir
import concourse.tile as tile
from concourse import bacc
from concourse.bass_utils import run_bass_kernel_spmd

F32 = mybir.dt.float32
BF16 = mybir.dt.bfloat16
FP8 = mybir.dt.float8e4
AF = mybir.ActivationFunctionType
OP = mybir.AluOpType
DR = mybir.MatmulPerfMode.DoubleRow
NP8 = ml_dtypes.float8_e4m3

B, C, H, W = 4, 256, 64, 64
N = H * W            # 4096 keys per batch
NCORES = 8
NH = N // 2          # 2048 queries per core
NBLK = 512           # query block
NBLOCKS = NH // NBLK
MT = N // 128        # 32 key tiles
MT2 = MT // 2        # 16 DoubleRow key-pair steps
YW = 272             # yt row: 256 channels + ones col + pad to %16
NT = NBLK // 128     # 4 query tiles per block
EPS = 1e-5
SCALE = float(C) ** -0.5
EOFF = 2.5           # exp offset; cancels in P'/S


def build():
    nc = bacc.Bacc("TRN2", target_bir_lowering=False, debug=False,
                   num_devices=NCORES)
    q8_d = nc.dram_tensor("q8", [128, 2 * NH], FP8, kind="ExternalInput")
    x2_d = nc.dram_tensor("x2dr", [128, 2 * N], FP8, kind="ExternalInput")
    yt_d = nc.dram_tensor("yt", [128, MT * YW], FP8, kind="ExternalInput")
    x1_d = nc.dram_tensor("x1t", [NH, C], BF16, kind="ExternalInput")
    gc_d = nc.dram_tensor("gatec", [128, NBLOCKS * NT], F32,
                          kind="ExternalInput")
    out_d = nc.dram_tensor("out", [NH, C], BF16, kind="ExternalOutput")

    with tile.TileContext(nc, pool_alloc_mode="queue") as tc, ExitStack() as ctx:
        pers = ctx.enter_context(tc.tile_pool(name="pers", bufs=1))
        epool = ctx.enter_context(tc.tile_pool(name="epool", bufs=2))
        work = ctx.enter_context(tc.tile_pool(name="work", bufs=2))
        outs = ctx.enter_context(tc.tile_pool(name="outs", bufs=2))
        psL = ctx.enter_context(tc.tile_pool(name="psL", bufs=3, space="PSUM"))
        psF = ctx.enter_context(tc.tile_pool(name="psF", bufs=2, space="PSUM"))

        q8b = [pers.tile([128, 2, NBLK], FP8, tag=f"q8b{j}", name=f"q8b{j}")
               for j in range(NBLOCKS)]
        # x2 chunks: small leading chunks (block-0 compute starts as soon as
        # its chunk lands — dep tracking is whole-tile) then two big ones
        # (fewer DMA triggers => fewer semaphores => shorter epilogue)
        x2s = [pers.tile([128, 2, 512], FP8, tag=f"x2s{c}", name=f"x2s{c}")
               for c in range(2)]
        x2b = [pers.tile([128, 2, 1536], FP8, tag=f"x2b{c}", name=f"x2b{c}")
               for c in range(2)]
        yts = [pers.tile([128, 4, YW], FP8, tag=f"yts{i}", name=f"yts{i}")
               for i in range(2)]
        ytb = [pers.tile([128, 6, YW], FP8, tag=f"ytb{i}", name=f"ytb{i}")
               for i in range(4)]
        x1ps = pers.tile([128, 4, C], BF16, tag="x1ps", name="x1ps")
        x1pb = pers.tile([128, NBLOCKS * NT - 4, C], BF16, tag="x1pb",
                         name="x1pb")
        gc = pers.tile([128, NBLOCKS * NT], F32, tag="gc", name="gc")
        nbias = pers.tile([128, 1], F32, tag="nbias", name="nbias")

        def _x1p(t):
            return x1ps[:, t, :] if t < 4 else x1pb[:, t - 4, :]

        def _x2ap(mt):
            # [128, 2, 128] lhsT slice for key tile mt
            if mt < 8:
                return x2s[mt // 4][:, :, (mt % 4) * 128:(mt % 4 + 1) * 128]
            c, o = (0, mt - 8) if mt < 20 else (1, mt - 20)
            return x2b[c][:, :, o * 128:(o + 1) * 128]

        def _ytap(k):
            # [128, 2, 257] rhs slice for fusion step k (key pair 2k,2k+1)
            t = 2 * k
            if t < 8:
                tile, o = yts[t // 4], t % 4
            else:
                tile, o = ytb[(t - 8) // 6], (t - 8) % 6
            return tile[:, o:o + 2, 0:257]

        def _x2dma(tile, c0, w):
            nc.sync.dma_start(tile[:, 0, :], x2_d[:, c0:c0 + w])
            nc.gpsimd.dma_start(tile[:, 1, :], x2_d[:, N + c0:N + c0 + w])

        def _ytdma(tile, t0, tn, eng):
            eng.dma_start(
                tile[:],
                yt_d[:, t0 * YW:(t0 + tn) * YW].rearrange(
                    "p (t y) -> p t y", y=YW))

        def _q8(j):
            js = slice(j * NBLK, (j + 1) * NBLK)
            nc.sync.dma_start(q8b[j][:, 0, :], q8_d[:, js])
            nc.gpsimd.dma_start(q8b[j][:, 1, :],
                                q8_d[:, NH + j * NBLK:NH + (j + 1) * NBLK])

        with nc.named_scope("pre"):
            # critical-path DMAs first, spread over three engine rings
            # (only sync/gpsimd/scalar can initiate DMAs): each SWDGE ring
            # has ~2us cold-start, and per-ring FIFO order matches first
            # use (x2s0+q8b0 gate the first logits, yts0 gates fusion k=0)
            nc.sync.dma_start(x2s[0][:, 0, :], x2_d[:, 0:512])
            nc.gpsimd.dma_start(x2s[0][:, 1, :], x2_d[:, N:N + 512])
            nc.scalar.dma_start(q8b[0][:, 0, :], q8_d[:, 0:NBLK])
            nc.sync.dma_start(q8b[0][:, 1, :], q8_d[:, NH:NH + NBLK])
            nc.gpsimd.dma_start(x2s[1][:, 0, :], x2_d[:, 512:1024])
            nc.scalar.dma_start(x2s[1][:, 1, :], x2_d[:, N + 512:N + 1024])
            _ytdma(yts[0], 0, 4, nc.scalar)
            # PE warmup: ramp the clock out of the low p-state on scratch
            # data while the first DMAs land; memsets on the (idle) DVE
            # queue so they don't delay the DMA triggers.
            wl = pers.tile([128, 2, 128], FP8, tag="wl", name="wl")
            wr = pers.tile([128, 2, 257], FP8, tag="wr", name="wr")
            nc.vector.memset(wl[:], 0.0)
            nc.vector.memset(wr[:], 0.0)
            nc.vector.memset(nbias[:], -EOFF)
            for w in range(12):
                wp = psF.tile([128, 257], F32, tag="fuse", name="fuse")
                nc.tensor.matmul(wp[:], wl[:], wr[:], start=True, stop=True,
                                 perf_mode=DR)

        def dma_feed():
            # Deferred DMA issues, a step per slot, just ahead of need;
            # per-ring FIFO order tracks each tile's first-use slot.
            nc.sync.dma_start(x2b[0][:, 0, :], x2_d[:, 1024:2560])
            nc.gpsimd.dma_start(x2b[0][:, 1, :], x2_d[:, N + 1024:N + 2560])
            yield
            _ytdma(ytb[0], 8, 6, nc.sync)
            _ytdma(yts[1], 4, 4, nc.gpsimd)
            yield
            nc.sync.dma_start(x2b[1][:, 0, :], x2_d[:, 2560:4096])
            _ytdma(ytb[1], 14, 6, nc.gpsimd)
            yield
            _ytdma(ytb[2], 20, 6, nc.sync)
            nc.gpsimd.dma_start(x2b[1][:, 1, :], x2_d[:, N + 2560:N + 4096])
            yield
            nc.sync.dma_start(q8b[1][:, 0, :], q8_d[:, NBLK:2 * NBLK])
            _ytdma(ytb[3], 26, 6, nc.gpsimd)
            yield
            nc.gpsimd.dma_start(q8b[1][:, 1, :],
                                q8_d[:, NH + NBLK:NH + 2 * NBLK])
            nc.sync.dma_start(
                x1ps[:], x1_d[0:512, :].rearrange("(t p) c -> p t c", p=128))
            yield
            nc.gpsimd.dma_start(gc[:], gc_d[:])
            yield
            nc.sync.dma_start(
                x1pb[:], x1_d[512:NH, :].rearrange("(t p) c -> p t c", p=128))
            _q8(2)
            yield
            _q8(3)
            yield

        feed = dma_feed()

        obufs = {}
        pcnt = {}
        back_q = []
        tail_mode = [False]

        def emit_post_front(j, nt, fuse_nt):
            # bank-freeing half of the postlude, straight off PSUM:
            # ginv=gate/S, rg=relu(P'*ginv).  (BN bias already in P'.)
            t_idx = j * NT + nt
            with nc.named_scope(f"post{j}_{nt}"):
                ginv = work.tile([128, 1], F32, tag="ginv", name="ginv",
                                 bufs=4)
                nc.vector.reciprocal_approx_fast(ginv[:],
                                                 fuse_nt[:, 256:257])
                nc.vector.tensor_scalar(ginv[:], ginv[:],
                                        gc[:, t_idx:t_idx + 1], None,
                                        op0=OP.mult)
                rg = work.tile([128, C], F32, tag="rg", name="rg", bufs=4)
                nc.vector.tensor_scalar(rg[:], fuse_nt[:, 0:256], ginv[:],
                                        0.0, op0=OP.mult, op1=OP.max)
            return rg

        def emit_post_back(j, nt, rg):
            # out=rg+x1; rows collect per block and leave as one DMA
            t_idx = j * NT + nt
            with nc.named_scope(f"postb{j}_{nt}"):
                if j not in obufs:
                    obufs[j] = outs.tile([128, NT, C], BF16, tag="ob",
                                         name="ob")
                    pcnt[j] = 0
                nc.vector.scalar_tensor_tensor(
                    obufs[j][:, nt, :], rg[:], 0.0, _x1p(t_idx),
                    op0=OP.bypass, op1=OP.add)
                pcnt[j] += 1
                if pcnt[j] == NT:
                    eng = nc.sync if j % 2 == 0 else nc.gpsimd
                    eng.dma_start(
                        out_d[j * NBLK:(j + 1) * NBLK, :].rearrange(
                            "(t p) c -> p t c", p=128),
                        obufs[j][:])

        def emit_fusion(e8, nt, k, fuse, fcnt, j):
            if fcnt[nt] == 0:
                if j == NBLOCKS - 1 and nt >= 2:
                    # tail B-pass: the logits ring is dead by now, so
                    # accumulate in psL banks — no wait on the psF bank
                    # chain behind pass A's posts
                    fb = psL.tile([128, 2, NBLK], F32, tag="L",
                                  name="fuseB")
                    fuse[nt] = fb[:, 0, :]
                else:
                    fuse[nt] = psF.tile([128, 257], F32, tag="fuse",
                                        name="fuse")
            ex = e8[k % 2]
            kk = (k // 2) * 2
            nc.tensor.matmul(
                fuse[nt][:, 0:257],
                ex[:, kk:kk + 2, nt * 128:(nt + 1) * 128],
                _ytap(k),
                start=(fcnt[nt] == 0), stop=(fcnt[nt] == MT2 - 1),
                perf_mode=DR)
            fcnt[nt] += 1
            if fcnt[nt] == MT2:
                rg = emit_post_front(j, nt, fuse[nt])
                if nt < 2 and not tail_mode[0]:
                    # pass-A backs complete in the next block's first
                    # slots; defer the out-row write so the bank-freeing
                    # fronts (which gate pass B) run first on DVE
                    back_q.append((j, nt, rg))
                else:
                    emit_post_back(j, nt, rg)

        # Hybrid exp, one instruction per slot.  ACT slots use the
        # activation table; DVE slots run Schraudolph directly in the
        # fp8e4m3 bit domain — one saturating tensor_scalar to uint8:
        # i8 = z*8/ln2 + (56 - c - EOFF*8/ln2).  Underflow clamps to 0
        # (= exp underflow, correct); overflow can't reach 255 for our
        # logit range.
        A_S = (8.0 / float(np.log(2.0))) * SCALE
        B_S = 56.0 - 0.30 - EOFF * (8.0 / float(np.log(2.0)))
        U8 = mybir.dt.uint8

        def emit_exp(e8, mt2, lp, on_act):
            ex = e8[mt2 % 2]
            kk = (mt2 // 2) * 2
            dst = ex[:, kk:kk + 2, :]
            if on_act:
                nc.scalar.activation(dst, lp[:], AF.Exp, scale=SCALE,
                                     bias=nbias[:])
            else:
                nc.vector.tensor_scalar(dst.bitcast(U8), lp[:], A_S, B_S,
                                        op0=OP.mult, op1=OP.add)

        # Two-pass fusion: with only 2 psF banks (pass A = nt 0,1 during
        # the block; pass B = nt 2,3 draining early next block off the
        # still-live e8 tiles), psL gets 3 buffers and the
        # exp(k) -> logits(k+2) -> exp(k+2) PSUM-WAR ring stops pacing the
        # kernel.  Emission is strictly pass-ordered (A(j) after B(j-1))
        # — the psF bank chain would deadlock the in-order PE queue
        # otherwise.
        CAP = 8
        LAG = 3
        fq = []
        avail_at = {}

        def pump(budget):
            while fq and budget > 0:
                e8x, nt, k, fusex, fcntx, jx = fq.pop(0)
                emit_fusion(e8x, nt, k, fusex, fcntx, jx)
                budget -= 1

        # DVE whole-slot exp schedule (rest on ACT).  Middle blocks keep
        # DVE free for slots 0-3 so the previous block's A-pass posts run
        # immediately and release the psF banks before pass B pops.
        DVE_KS = {0: (1, 3, 5, 7, 9, 11, 13),
                  NBLOCKS - 1: (4, 6, 8, 10, 12, 14)}
        DVE_MID = (4, 6, 8, 10, 12, 14)

        S = 0
        prevB = []
        for j in range(NBLOCKS):
            with nc.named_scope(f"blk{j}"):
                e8 = [epool.tile([128, MT2, NBLK], FP8, tag=f"E8{p}",
                                 name=f"E8{p}") for p in range(2)]
                fuse = {}
                fcnt = [0] * NT
                myB = []
                dve_ks = DVE_KS.get(j, DVE_MID)
                for mt2 in range(MT2):
                    # fusion first: logits stall on the psL ring (WAR vs
                    # exp(k-3)) and the in-order PE queue would
                    # head-of-line-block queued fusion work behind them
                    if mt2 == 2:
                        # all A(j-1) items are enqueued by slot 1 (the
                        # last two use lag 2); B(j-1) goes next (before
                        # this block's own A items), nt-major so its
                        # banks free early
                        prevB.sort(key=lambda it: (it[1], it[2]))
                        fq.extend(prevB)
                        prevB = []
                    if mt2 == 4 and back_q:
                        # deferred pass-A out-row writes, now that the
                        # bank-freeing fronts have run
                        for jb, ntb, rgb in back_q:
                            emit_post_back(jb, ntb, rgb)
                        back_q.clear()
                    for item in avail_at.pop(S, ()):
                        fq.append(item)
                    pump(CAP)
                    lp = psL.tile([128, 2, NBLK], F32, tag="L", name="L")
                    for sub in range(2):
                        nc.tensor.matmul(
                            lp[:, sub, :], _x2ap(2 * mt2 + sub), q8b[j][:],
                            start=True, stop=True, perf_mode=DR)
                    emit_exp(e8, mt2, lp, mt2 not in dve_ks)
                    lag = LAG if mt2 < 14 else 2
                    avail_at.setdefault(S + lag, []).extend(
                        (e8, nt, mt2, fuse, fcnt, j) for nt in (0, 1))
                    myB.extend((e8, nt, mt2, fuse, fcnt, j) for nt in (2, 3))
                    next(feed, None)
                    S += 1
                prevB = myB
        with nc.named_scope("tail"):
            # all exps are emitted; drain remaining A items then the last
            # B pass, nt-major so each nt's post (and its out-DMA) fires
            # as early as possible
            tail_mode[0] = True
            for jb, ntb, rgb in back_q:
                emit_post_back(jb, ntb, rgb)
            back_q.clear()
            for items in avail_at.values():
                fq.extend(items)
            avail_at.clear()
            fq.sort(key=lambda it: (it[5], it[1], it[2]))
            prevB.sort(key=lambda it: (it[1], it[2]))
            fq.extend(prevB)
            pump(len(fq))
    nc.compile()
    return nc


_NC = None


def _get_nc():
    global _NC
    if _NC is None:
        _NC = build()
    return _NC


def kernel(**inputs):
    x1 = np.asarray(inputs["x1"], np.float32).reshape(B, C, N)
    x2 = np.asarray(inputs["x2"], np.float32).reshape(B, C, N)
    q_w = np.asarray(inputs["q_w"], np.float32)
    k_w = np.asarray(inputs["k_w"], np.float32)
    v_w = np.asarray(inputs["v_w"], np.float32)
    p_w = np.asarray(inputs["proj_w"], np.float32)
    q_b = np.asarray(inputs["q_b"], np.float32)
    v_b = np.asarray(inputs["v_b"], np.float32)
    p_b = np.asarray(inputs["proj_b"], np.float32)
    gamma = np.asarray(inputs["bn_gamma"], np.float32)
    beta = np.asarray(inputs["bn_beta"], np.float32)
    mean = np.asarray(inputs["bn_mean"], np.float32)
    var = np.asarray(inputs["bn_var"], np.float32)
    gate_w = np.asarray(inputs["gate_w"], np.float32)
    gate_b = np.asarray(inputs["gate_b"], np.float32)

    wqk = q_w.T @ k_w                      # [C,C]
    A = gamma[:, None] / np.sqrt(var + EPS)[:, None] * (p_w @ v_w)  # G*(pw vw)
    G = gamma / np.sqrt(var + EPS)
    Bc = (beta + (p_b + p_w @ v_b - mean) * G).astype(np.float32)
    qpb = (k_w.T @ q_b).astype(np.float32)

    in_maps = []
    for b in range(B):
        Qp = (wqk.T @ x1[b] + qpb[:, None]).astype(NP8)      # [C, N]
        Y8 = (A @ x2[b] + Bc[:, None]).astype(NP8)            # [C, N] + bias
        glog = gate_w[0, :C] @ x1[b] + gate_w[0, C:] @ x2[b] + gate_b[0]
        gate = (1.0 / (1.0 + np.exp(-glog))).astype(np.float32)  # [N]
        x28 = x2[b].astype(NP8)
        # x2dr/q8 layouts: [p, h, m] = arr[h*128+p, m]
        x2dr = np.ascontiguousarray(
            x28.reshape(2, 128, N).transpose(1, 0, 2).reshape(128, 2 * N))
        yt = np.zeros((128, MT, YW), NP8)
        yt[:, :, :C] = np.ascontiguousarray(
            Y8.reshape(C, MT, 128).transpose(2, 1, 0))
        yt[:, :, C] = np.float32(1.0)
        yt = np.ascontiguousarray(yt.reshape(128, MT * YW))
        for half in range(2):
            hq = slice(half * NH, (half + 1) * NH)
            q8 = np.ascontiguousarray(
                Qp[:, hq].reshape(2, 128, NH).transpose(1, 0, 2)
                .reshape(128, 2 * NH))
            x1t = np.ascontiguousarray(
                x1[b][:, hq].T.astype(ml_dtypes.bfloat16))    # [NH, C]
            gc = np.ascontiguousarray(
                gate[hq].reshape(NBLOCKS * NT, 128).T.astype(np.float32))
            in_maps.append({
                "q8": q8, "x2dr": x2dr, "yt": yt, "x1t": x1t,
                "gatec": gc,
            })

    nc = _get_nc()
    res = run_bass_kernel_spmd(nc, in_maps, core_ids=list(range(NCORES)))
    out = np.empty((B, C, N), np.float32)
    for core in range(NCORES):
        b, half = divmod(core, 2)
        out[b, :, half * NH:(half + 1) * NH] = \
            res.results[core]["out"].astype(np.float32).T
    return out.reshape(B, C, H, W)
